# revision 14
# baseline (speedup 1.0000x reference)
"""Trainium2 Bass kernel for an 11-stage butterfly linear layer + bias.

Problem: x (16384, 2048) fp32; out[b, :] = B @ x[b, :] + bias where B is the
composition of 11 butterfly stages (strides 1..1024), each an elementwise 2x2
mix of position pairs with learned per-pair coefficients.

Factorization (positions p = blk*128 + w, blk in [0,16), w in [0,128)):
  - Stages 0-6 (strides 1..64) mix within a 128-block -> block-diagonal
    D = diag(D_0..D_15), each 128x128 dense.
  - Stages 7-10 (strides 128..1024) mix across blocks, separately per w ->
    per-w 16x16 matrices C_w.  Grouping q = w8*16 + b over w-group t = w//8
    makes this block-diagonal too (128x128 per group t).

v5 design (185 us baseline -> 148 v2 -> 126 v4 -> this):
  - x pre-transposed on the HOST into per-chunk [pos, block, batch] layout:
    zero TensorE transposes, contiguous input DMA.  bf16 output (host casts
    back to f32): per-core HBM = 8 MiB in + 8 MiB out ~= 47 us roofline.
  - W1 columns carry sigma(t*8+w8) = 32*(t//4) + t%4 + 4*w8 so each w-group
    lives on stride-4 partitions = 8 distinct SBUF AXI ports: the per-t mid
    permute gather reads at fabric rate instead of 2 ports (v2/v3 disease).
  - CH=512, 4 chunks/core, PE stream explicitly INTERLEAVED: [MM1(c) b-pair,
    MM2(c-1) t-octet, ...] so ACT drains (chunk c) and DVE bias-adds (chunk
    c-1) run concurrently and the PE never idles past the HAM window (v4
    alternated whole phases: each engine idled half the time, PE ran at
    K=4/8 for 65% of the kernel).
  - Permute issues spread over sync/scalar/gpsimd so no ring exceeds the
    11.7 us/chunk HBM pace; drains split 6 ACT / 2 DVE.
"""

import sys

import numpy as np

sys.path.insert(0, "/opt/trn_rl_repo")

import concourse.bass as bass  # noqa: E402
import concourse.mybir as mybir  # noqa: E402
import concourse.tile as tile  # noqa: E402
from concourse import bacc  # noqa: E402
from concourse.bass import ds, ts  # noqa: E402
from concourse.bass_utils import run_bass_kernel_spmd  # noqa: E402

N = 2048
LOG_N = 11
NCORES = 8
BATCH = 16384
BPC = BATCH // NCORES  # batch rows per core
P = 128
NB = 16  # number of 128-blocks
CH = 512  # batch rows per pipeline chunk
CHN = BPC // CH  # chunks per core
HQ = CH // P  # 128-row groups per chunk

WARMUP_MMS = 16  # PE warmup matmuls (N=256) on a memset tile

PROFILE = False
LAST_RESULTS = None

_NC_CACHE = {}


def _emit_body(ctx, tc, aps):
    nc = tc.nc
    x_ap, w1_ap, c2_ap, bb_ap, out_ap = aps
    f32 = mybir.dt.float32
    bf16 = mybir.dt.bfloat16

    const = ctx.enter_context(tc.tile_pool(name="const", bufs=1))
    W1 = const.tile([P, NB * P], bf16)
    C2 = const.tile([P, NB * P], bf16)
    BB = const.tile([P, N], bf16)
    nc.scalar.dma_start(W1[:], w1_ap)
    nc.scalar.dma_start(C2[:], c2_ap)
    nc.scalar.dma_start(BB[:], bb_ap)

    xpool = ctx.enter_context(tc.tile_pool(name="xin", bufs=3))
    ypool = ctx.enter_context(tc.tile_pool(name="ymid", bufs=2))
    yppool = ctx.enter_context(tc.tile_pool(name="ypmid", bufs=2 * NB))
    opool = ctx.enter_context(tc.tile_pool(name="oout", bufs=6))
    ps_m1 = ctx.enter_context(tc.tile_pool(name="ps_m1", bufs=2, space="PSUM"))
    ps_m2 = ctx.enter_context(tc.tile_pool(name="ps_m2", bufs=2, space="PSUM"))

    xts = []
    for c in range(CHN):
        xts.append(xpool.tile([P, NB * CH], bf16, name=f"A_{c}", tag="A"))

    def dma_in(c):
        nc.sync.dma_start(xts[c][:], x_ap[c * P : (c + 1) * P, :])

    dma_in(0)
    dma_in(1)

    # ---- PE warmup on a memset tile: no DMA dependency, starts immediately ----
    wt = const.tile([P, 2 * P], bf16)
    nc.vector.memset(wt[:], 1.0)
    wps = ps_m2.tile([P, 2 * CH], f32, name="warm", tag="pz")
    for i in range(WARMUP_MMS):
        nc.tensor.matmul(
            wps[:, ds(0, 256)], wt[:, ts(0, P)], wt[:], start=True, stop=True
        )

    ypss = {}

    def mm1_pair(c, bp, Ysb):
        """2 matmuls (b-pair, N=512 each) into a 2-bank PSUM tile + drain."""
        A = xts[c]
        pp = ps_m1.tile([P, 2 * CH], f32, name=f"pp_{c}_{bp}", tag="pp")
        for i in range(2):
            b = bp * 2 + i
            nc.tensor.matmul(
                pp[:, ts(i, CH)],
                W1[:, ts(b, P)],
                A[:, ts(b, CH)],
                start=True,
                stop=True,
            )
        if bp % 4 == 3:
            nc.vector.tensor_copy(Ysb[:, ds(bp * 2 * CH, 2 * CH)], pp[:])
        else:
            nc.scalar.copy(Ysb[:, ds(bp * 2 * CH, 2 * CH)], pp[:])

    def permutes(c, Ysb):
        """Per-t gathers: Yp_t[w8*16+b, f] = Ysb[sigma(t,w8), b*CH+f].

        sigma spreads the 8 source partitions of each t over 8 distinct AXI
        ports (stride 4).  Issues rotate over sync/scalar/gpsimd rings.
        """
        srcv = Ysb[:].rearrange(
            "(B w r) (b f) -> B r w b f", B=4, w=8, r=4, b=NB, f=CH
        )
        yps = [None] * NB
        engs = [nc.sync, nc.scalar, nc.gpsimd]
        for i, t in enumerate(x for u in range(8) for x in (u, u + 8)):
            Yp = yppool.tile([P, CH], bf16, name=f"Yp_{c}_{t}", tag="Yp")
            engs[i % 3].dma_start(Yp[:], srcv[t // 4, t % 4])
            yps[t] = Yp
        ypss[c] = yps

    def mm2_group(c, g):
        """One t-octet of MM2 for row-group hh: 8 matmuls + DVE bias-add."""
        hh, tp = divmod(g, 2)
        yps = ypss[c]
        O = ypss.setdefault(("O", c, hh), None)
        if O is None:
            O = opool.tile([P, N], bf16, name=f"O_{c}_{hh}", tag="O")
            ypss[("O", c, hh)] = O
        pz = ps_m2.tile([P, 8 * P], f32, name=f"pz_{c}_{hh}_{tp}", tag="pz")
        for j in range(8):
            t = tp * 8 + j
            nc.tensor.matmul(
                pz[:, ts(j, P)],
                yps[t][:, ts(hh, P)],
                C2[:, ts(t, P)],
                start=True,
                stop=True,
            )
        dsto = O[:].rearrange("p (b t w) -> p b t w", b=16, t=16, w=8)[
            :, :, tp * 8 : (tp + 1) * 8, :
        ]
        src = pz[:].rearrange("p (t b w) -> p b t w", t=8, b=16, w=8)
        bsrc = BB[:].rearrange("p (b t w) -> p b t w", b=16, t=16, w=8)[
            :, :, tp * 8 : (tp + 1) * 8, :
        ]
        nc.vector.tensor_add(dsto, src, bsrc)
        if tp == 1:
            nc.sync.dma_start(
                out_ap[c * CH + hh * P : c * CH + (hh + 1) * P, :], O[:]
            )

    # ---- software-pipelined, PE-interleaved emission ----
    for c in range(CHN):
        Ysb = ypool.tile([P, NB * CH], bf16, name=f"Ysb_{c}", tag="Ysb")
        if c + 1 < CHN:
            dma_in(c + 1)
        for i in range(8):
            mm1_pair(c, i, Ysb)
            if c >= 1:
                mm2_group(c - 1, i)
        permutes(c, Ysb)
    for g in range(8):
        mm2_group(CHN - 1, g)


def build_nc():
    nc = bacc.Bacc(
        "TRN2",
        target_bir_lowering=False,
        debug=False,
        num_devices=NCORES,
    )
    x_ap = nc.dram_tensor(
        "x", [CHN * P, NB * CH], mybir.dt.bfloat16, kind="ExternalInput"
    ).ap()
    w1_ap = nc.dram_tensor("w1", [P, NB * P], mybir.dt.bfloat16, kind="ExternalInput").ap()
    c2_ap = nc.dram_tensor("c2", [P, NB * P], mybir.dt.bfloat16, kind="ExternalInput").ap()
    bb_ap = nc.dram_tensor("bb", [P, N], mybir.dt.bfloat16, kind="ExternalInput").ap()
    out_ap = nc.dram_tensor("out", [BPC, N], mybir.dt.bfloat16, kind="ExternalOutput").ap()

    from contextlib import ExitStack

    with tile.TileContext(nc) as tc:
        with ExitStack() as ctx:
            _emit_body(ctx, tc, (x_ap, w1_ap, c2_ap, bb_ap, out_ap))
    nc.compile()
    return nc


def _butterfly_apply(tw, X, idx_lo, idx_hi):
    """Apply butterfly stages [idx_lo, idx_hi) to rows of X. tw: (LOG_N, N//2, 2, 2)."""
    out = X
    for idx in range(idx_lo, idx_hi):
        s = 1 << idx
        g = N // (2 * s)
        T = tw[idx].reshape(g, s, 2, 2)
        xr = out.reshape(-1, g, 2, s)
        out = np.einsum("gsij,bgjs->bgis", T, xr).reshape(-1, N)
    return out


def host_weights(twiddle, bias):
    """Build device constants from the twiddle/bias arrays."""
    import ml_dtypes

    tw = np.asarray(twiddle, dtype=np.float64)[0, 0]  # (LOG_N, N//2, 2, 2)
    eye = np.eye(N, dtype=np.float64)
    R1 = _butterfly_apply(tw, eye, 0, 7)  # = D^T, block-diagonal
    R2 = _butterfly_apply(tw, eye, 7, LOG_N)  # = C^T

    # W1 lhsT per block b: lhsT[p, sigma(w)] = D_b[w, p] = R1 block (b, b).
    # sigma(t*8+w8) = 32*(t//4) + t%4 + 4*w8 spreads each w-group over the
    # SBUF AXI ports so the mid permute reads at full fabric rate.
    w = np.arange(P)
    sigma = 32 * (w // 8 // 4) + (w // 8) % 4 + 4 * (w % 8)
    w1 = np.zeros((P, NB * P))
    for b in range(NB):
        w1[:, b * P + sigma] = R1[b * P : (b + 1) * P, b * P : (b + 1) * P]
    # C2 lhsT per w-group t: rows q = w8*16+b (mid pos), cols j = bo*8+wo8 (out pos)
    c2 = np.zeros((P, NB * P))
    q = np.arange(P)
    for t in range(NB):
        pm = (q % 16) * P + t * 8 + (q // 16)
        pn = (q // 8) * P + t * 8 + (q % 8)
        c2[:, t * P : (t + 1) * P] = R2[np.ix_(pm, pn)]
    bb = np.broadcast_to(np.asarray(bias, dtype=np.float64)[None, :], (P, N))
    return (
        np.ascontiguousarray(w1.astype(ml_dtypes.bfloat16)),
        np.ascontiguousarray(c2.astype(ml_dtypes.bfloat16)),
        np.ascontiguousarray(bb.astype(ml_dtypes.bfloat16)),
    )


def host_x(x):
    """bf16-cast + per-core chunked transpose: [c][fc][p][b][f] layout."""
    import ml_dtypes

    xb = np.asarray(x).astype(ml_dtypes.bfloat16)
    # rows = c*2048 + fc*CH + f; cols = b*128 + p
    xr = xb.reshape(NCORES, CHN, CH, NB, P).transpose(0, 1, 4, 3, 2)
    return np.ascontiguousarray(xr)  # (8, CHN, 128, 16, CH)


def kernel(x, twiddle, bias):
    global LAST_RESULTS

    assert x.shape == (BATCH, N), x.shape

    if "nc" not in _NC_CACHE:
        _NC_CACHE["nc"] = build_nc()
    nc = _NC_CACHE["nc"]

    w1, c2, bb = host_weights(twiddle, bias)
    xr = host_x(x)
    in_maps = [
        {
            "x": xr[c].reshape(CHN * P, NB * CH),
            "w1": w1,
            "c2": c2,
            "bb": bb,
        }
        for c in range(NCORES)
    ]
    res = run_bass_kernel_spmd(
        nc, in_maps, core_ids=list(range(NCORES)), trace=PROFILE
    )
    LAST_RESULTS = res
    out = np.concatenate([res.results[c]["out"] for c in range(NCORES)], axis=0)
    return out.astype(np.float32)


# revision 17
# speedup vs baseline: 1.1451x; 1.1451x over previous
"""Trainium2 Bass kernel for an 11-stage butterfly linear layer + bias.

Problem: x (16384, 2048) fp32; out[b, :] = B @ x[b, :] + bias where B is the
composition of 11 butterfly stages (strides 1..1024), each an elementwise 2x2
mix of position pairs with learned per-pair coefficients.

Factorization (positions p = blk*128 + w, blk in [0,16), w in [0,128)):
  - Stages 0-6 (strides 1..64) mix within a 128-block -> block-diagonal
    D = diag(D_0..D_15), each 128x128 dense.
  - Stages 7-10 (strides 128..1024) mix across blocks, separately per w ->
    per-w 16x16 matrices C_w.  Grouping q = w8*16 + b over w-group t = w//8
    makes this block-diagonal too (128x128 per group t).

v5 design (185 us baseline -> 148 v2 -> 126 v4 -> this):
  - x pre-transposed on the HOST into per-chunk [pos, block, batch] layout:
    zero TensorE transposes, contiguous input DMA.  bf16 output (host casts
    back to f32): per-core HBM = 8 MiB in + 8 MiB out ~= 47 us roofline.
  - W1 columns carry sigma(t*8+w8) = 32*(t//4) + t%4 + 4*w8 so each w-group
    lives on stride-4 partitions = 8 distinct SBUF AXI ports: the per-t mid
    permute gather reads at fabric rate instead of 2 ports (v2/v3 disease).
  - CH=512, 4 chunks/core, PE stream explicitly INTERLEAVED: [MM1(c) b-pair,
    MM2(c-1) t-octet, ...] so ACT drains (chunk c) and DVE bias-adds (chunk
    c-1) run concurrently and the PE never idles past the HAM window (v4
    alternated whole phases: each engine idled half the time, PE ran at
    K=4/8 for 65% of the kernel).
  - Permute issues spread over sync/scalar/gpsimd so no ring exceeds the
    11.7 us/chunk HBM pace; drains split 6 ACT / 2 DVE.
"""

import sys

import numpy as np

sys.path.insert(0, "/opt/trn_rl_repo")

import concourse.bass as bass  # noqa: E402
import concourse.mybir as mybir  # noqa: E402
import concourse.tile as tile  # noqa: E402
from concourse import bacc  # noqa: E402
from concourse.bass import ds, ts  # noqa: E402
from concourse.bass_utils import run_bass_kernel_spmd  # noqa: E402

N = 2048
LOG_N = 11
NCORES = 8
BATCH = 16384
BPC = BATCH // NCORES  # batch rows per core
P = 128
NB = 16  # number of 128-blocks
CH = 512  # batch rows per pipeline chunk
CHN = BPC // CH  # chunks per core
HQ = CH // P  # 128-row groups per chunk

WARMUP_MMS = 16  # PE warmup matmuls (N=256) on a memset tile

PROFILE = False
LAST_RESULTS = None

_NC_CACHE = {}


def _emit_body(ctx, tc, aps):
    nc = tc.nc
    x_ap, w1_ap, c2_ap, bb_ap, out_ap = aps
    f32 = mybir.dt.float32
    bf16 = mybir.dt.bfloat16

    const = ctx.enter_context(tc.tile_pool(name="const", bufs=1))
    W1 = const.tile([P, NB * P], bf16)
    C2 = const.tile([P, NB * P], bf16)
    BB = const.tile([P, N], bf16)
    nc.scalar.dma_start(W1[:], w1_ap)
    nc.scalar.dma_start(C2[:], c2_ap)
    nc.scalar.dma_start(BB[:], bb_ap)

    xpool = ctx.enter_context(tc.tile_pool(name="xin", bufs=3))
    ypool = ctx.enter_context(tc.tile_pool(name="ymid", bufs=2))
    yppool = ctx.enter_context(tc.tile_pool(name="ypmid", bufs=3 * NB))
    opool = ctx.enter_context(tc.tile_pool(name="oout", bufs=8))
    ps_m1 = ctx.enter_context(tc.tile_pool(name="ps_m1", bufs=2, space="PSUM"))
    ps_m2 = ctx.enter_context(tc.tile_pool(name="ps_m2", bufs=2, space="PSUM"))

    xts = []
    for c in range(CHN):
        xts.append(xpool.tile([P, NB * CH], bf16, name=f"A_{c}", tag="A"))

    def dma_in(c):
        nc.sync.dma_start(xts[c][:], x_ap[c * P : (c + 1) * P, :])

    dma_in(0)
    dma_in(1)

    # ---- PE warmup on a memset tile: no DMA dependency, starts immediately ----
    wt = const.tile([P, 2 * P], bf16)
    nc.vector.memset(wt[:], 1.0)
    wps = ps_m2.tile([P, 2 * CH], f32, name="warm", tag="pz")
    for i in range(WARMUP_MMS):
        nc.tensor.matmul(
            wps[:, ds(0, 256)], wt[:, ts(0, P)], wt[:], start=True, stop=True
        )

    ypss = {}

    def mm1_pair(c, bp, Ysb):
        """2 matmuls (b-pair, N=512 each) into a 2-bank PSUM tile + drain."""
        A = xts[c]
        pp = ps_m1.tile([P, 2 * CH], f32, name=f"pp_{c}_{bp}", tag="pp")
        for i in range(2):
            b = bp * 2 + i
            nc.tensor.matmul(
                pp[:, ts(i, CH)],
                W1[:, ts(b, P)],
                A[:, ts(b, CH)],
                start=True,
                stop=True,
            )
        # chunks 0-1 run without interleaved MM2 work (pipeline fill): DVE is
        # idle there, so split drains 4/4; steady-state chunks go 6/2.
        dve = bp % 2 == 1 if c < 2 else bp % 4 == 3
        if dve:
            nc.vector.tensor_copy(Ysb[:, ds(bp * 2 * CH, 2 * CH)], pp[:])
        else:
            nc.scalar.copy(Ysb[:, ds(bp * 2 * CH, 2 * CH)], pp[:])

    def permutes(c, Ysb):
        """Per-t gathers: Yp_t[w8*16+b, f] = Ysb[sigma(t,w8), b*CH+f].

        sigma spreads the 8 source partitions of each t over 8 distinct AXI
        ports (stride 4).  Issues rotate over sync/scalar/gpsimd rings.
        """
        srcv = Ysb[:].rearrange(
            "(B w r) (b f) -> B r w b f", B=4, w=8, r=4, b=NB, f=CH
        )
        yps = [None] * NB
        engs = [nc.sync, nc.scalar, nc.gpsimd]
        for i, t in enumerate(x for u in range(8) for x in (u, u + 8)):
            Yp = yppool.tile([P, CH], bf16, name=f"Yp_{c}_{t}", tag="Yp")
            engs[i % 3].dma_start(Yp[:], srcv[t // 4, t % 4])
            yps[t] = Yp
        ypss[c] = yps

    def mm2_group(c, g):
        """One t-octet of MM2 for row-group hh: 8 matmuls + DVE bias-add."""
        hh, tp = divmod(g, 2)
        yps = ypss[c]
        O = ypss.setdefault(("O", c, hh), None)
        if O is None:
            O = opool.tile([P, N], bf16, name=f"O_{c}_{hh}", tag="O")
            ypss[("O", c, hh)] = O
        pz = ps_m2.tile([P, 8 * P], f32, name=f"pz_{c}_{hh}_{tp}", tag="pz")
        for j in range(8):
            t = tp * 8 + j
            nc.tensor.matmul(
                pz[:, ts(j, P)],
                yps[t][:, ts(hh, P)],
                C2[:, ts(t, P)],
                start=True,
                stop=True,
            )
        dsto = O[:].rearrange("p (b t w) -> p b t w", b=16, t=16, w=8)[
            :, :, tp * 8 : (tp + 1) * 8, :
        ]
        src = pz[:].rearrange("p (t b w) -> p b t w", t=8, b=16, w=8)
        bsrc = BB[:].rearrange("p (b t w) -> p b t w", b=16, t=16, w=8)[
            :, :, tp * 8 : (tp + 1) * 8, :
        ]
        nc.vector.tensor_add(dsto, src, bsrc)
        if tp == 1:
            nc.sync.dma_start(
                out_ap[c * CH + hh * P : c * CH + (hh + 1) * P, :], O[:]
            )

    # ---- software-pipelined, PE-interleaved emission (2-chunk lookahead:
    # MM2 of chunk c-2 interleaves into chunk c so the drain->permute barrier
    # of c-2 is fully hidden and the PE stream never head-blocks) ----
    for c in range(CHN):
        Ysb = ypool.tile([P, NB * CH], bf16, name=f"Ysb_{c}", tag="Ysb")
        if c + 1 < CHN:
            dma_in(c + 1)
        for i in range(8):
            mm1_pair(c, i, Ysb)
            if c >= 2:
                mm2_group(c - 2, i)
        permutes(c, Ysb)
    for c in (CHN - 2, CHN - 1):
        for g in range(8):
            mm2_group(c, g)


def build_nc():
    nc = bacc.Bacc(
        "TRN2",
        target_bir_lowering=False,
        debug=False,
        num_devices=NCORES,
    )
    x_ap = nc.dram_tensor(
        "x", [CHN * P, NB * CH], mybir.dt.bfloat16, kind="ExternalInput"
    ).ap()
    w1_ap = nc.dram_tensor("w1", [P, NB * P], mybir.dt.bfloat16, kind="ExternalInput").ap()
    c2_ap = nc.dram_tensor("c2", [P, NB * P], mybir.dt.bfloat16, kind="ExternalInput").ap()
    bb_ap = nc.dram_tensor("bb", [P, N], mybir.dt.bfloat16, kind="ExternalInput").ap()
    out_ap = nc.dram_tensor("out", [BPC, N], mybir.dt.bfloat16, kind="ExternalOutput").ap()

    from contextlib import ExitStack

    with tile.TileContext(nc) as tc:
        with ExitStack() as ctx:
            _emit_body(ctx, tc, (x_ap, w1_ap, c2_ap, bb_ap, out_ap))
    nc.compile()
    return nc


def _butterfly_apply(tw, X, idx_lo, idx_hi):
    """Apply butterfly stages [idx_lo, idx_hi) to rows of X. tw: (LOG_N, N//2, 2, 2)."""
    out = X
    for idx in range(idx_lo, idx_hi):
        s = 1 << idx
        g = N // (2 * s)
        T = tw[idx].reshape(g, s, 2, 2)
        xr = out.reshape(-1, g, 2, s)
        out = np.einsum("gsij,bgjs->bgis", T, xr).reshape(-1, N)
    return out


def host_weights(twiddle, bias):
    """Build device constants from the twiddle/bias arrays."""
    import ml_dtypes

    tw = np.asarray(twiddle, dtype=np.float64)[0, 0]  # (LOG_N, N//2, 2, 2)
    eye = np.eye(N, dtype=np.float64)
    R1 = _butterfly_apply(tw, eye, 0, 7)  # = D^T, block-diagonal
    R2 = _butterfly_apply(tw, eye, 7, LOG_N)  # = C^T

    # W1 lhsT per block b: lhsT[p, sigma(w)] = D_b[w, p] = R1 block (b, b).
    # sigma(t*8+w8) = 32*(t//4) + t%4 + 4*w8 spreads each w-group over the
    # SBUF AXI ports so the mid permute reads at full fabric rate.
    w = np.arange(P)
    sigma = 32 * (w // 8 // 4) + (w // 8) % 4 + 4 * (w % 8)
    w1 = np.zeros((P, NB * P))
    for b in range(NB):
        w1[:, b * P + sigma] = R1[b * P : (b + 1) * P, b * P : (b + 1) * P]
    # C2 lhsT per w-group t: rows q = w8*16+b (mid pos), cols j = bo*8+wo8 (out pos)
    c2 = np.zeros((P, NB * P))
    q = np.arange(P)
    for t in range(NB):
        pm = (q % 16) * P + t * 8 + (q // 16)
        pn = (q // 8) * P + t * 8 + (q % 8)
        c2[:, t * P : (t + 1) * P] = R2[np.ix_(pm, pn)]
    bb = np.broadcast_to(np.asarray(bias, dtype=np.float64)[None, :], (P, N))
    return (
        np.ascontiguousarray(w1.astype(ml_dtypes.bfloat16)),
        np.ascontiguousarray(c2.astype(ml_dtypes.bfloat16)),
        np.ascontiguousarray(bb.astype(ml_dtypes.bfloat16)),
    )


def host_x(x):
    """bf16-cast + per-core chunked transpose: [c][fc][p][b][f] layout."""
    import ml_dtypes

    xb = np.asarray(x).astype(ml_dtypes.bfloat16)
    # rows = c*2048 + fc*CH + f; cols = b*128 + p
    xr = xb.reshape(NCORES, CHN, CH, NB, P).transpose(0, 1, 4, 3, 2)
    return np.ascontiguousarray(xr)  # (8, CHN, 128, 16, CH)


def kernel(x, twiddle, bias):
    global LAST_RESULTS

    assert x.shape == (BATCH, N), x.shape

    if "nc" not in _NC_CACHE:
        _NC_CACHE["nc"] = build_nc()
    nc = _NC_CACHE["nc"]

    w1, c2, bb = host_weights(twiddle, bias)
    xr = host_x(x)
    in_maps = [
        {
            "x": xr[c].reshape(CHN * P, NB * CH),
            "w1": w1,
            "c2": c2,
            "bb": bb,
        }
        for c in range(NCORES)
    ]
    res = run_bass_kernel_spmd(
        nc, in_maps, core_ids=list(range(NCORES)), trace=PROFILE
    )
    LAST_RESULTS = res
    out = np.concatenate([res.results[c]["out"] for c in range(NCORES)], axis=0)
    return out.astype(np.float32)
